# revision 19
# baseline (speedup 1.0000x reference)
"""Trainium2 Bass kernel for GQA multi-head attention (B=2,S=2048,HID=2048,H=32,KVH=8,D=64).

Sharding: 8 cores = 2 (batch) x 4 (kv-head groups). Each core handles one batch
element and 2 kv heads (= 8 q heads), computes its partial o_proj output
(contracting only its 512 attention features), host sums 4 partials per batch.

Pipeline layout (all GEMM operands bf16, f32 PSUM accumulate):
  prologue: K/V projections + K RoPE + V transpose, hs streamed once per chunk
  per 512-token chunk: Q proj + RoPE -> per j (4 q-tile pairs):
      scores (row-tiled 64+64 on the PE for both heads) -> exp (scalar engine,
      1024-wide, bf16 out) -> attnV (head A psum partitions 0:65 with rowsum
      at 64, head B partitions 63:128 with rowsum at 63 -- ones column FIRST
      in B's lhsT so no cross-partition shuffle is needed afterwards)
      -> reciprocal + PE outer-product broadcast -> normalized oT (bf16)
    then o_proj for the chunk + bf16 partial out DMA.

Softmax: scores are O(10) so exp without max-subtraction is safe; rowsums ride
as ones columns in the attn@V weights.
"""

import sys

if "/opt/trn_rl_repo" not in sys.path:
    sys.path.insert(0, "/opt/trn_rl_repo")

import numpy as np

B, S, HID = 2, 2048, 2048
H, KVH, D = 32, 8, 64
NCORES = 8

PERM_LOCAL = [0, 4, 1, 5, 2, 6, 3, 7]

_NC_CACHE = {}


def _build_nc(repeat=1):
    import concourse.bass as bass
    import concourse.mybir as mybir
    from concourse import bacc
    from concourse.tile import TileContext
    from concourse.masks import make_identity
    from contextlib import ExitStack

    f32 = mybir.dt.float32
    f32r = mybir.dt.float32r
    bf16 = mybir.dt.bfloat16
    Exp = mybir.ActivationFunctionType.Exp
    mult = mybir.AluOpType.mult
    add = mybir.AluOpType.add

    nc = bacc.Bacc(None, target_bir_lowering=False)

    hsT = nc.declare_dram_parameter("hsT", [HID, S], bf16, isOutput=False)
    cosT2 = nc.declare_dram_parameter("cosT2", [128, S], f32, isOutput=False)
    sinT2 = nc.declare_dram_parameter("sinT2", [128, S], f32, isOutput=False)
    rotm = nc.declare_dram_parameter("rotm", [128, 128], bf16, isOutput=False)
    wqT = nc.declare_dram_parameter("wqT", [HID, 512], bf16, isOutput=False)
    wkT = nc.declare_dram_parameter("wkT", [HID, 128], bf16, isOutput=False)
    wvT = nc.declare_dram_parameter("wvT", [HID, 128], bf16, isOutput=False)
    woT = nc.declare_dram_parameter("woT", [512, HID], bf16, isOutput=False)
    out = nc.declare_dram_parameter("out", [S, HID], bf16, isOutput=True)

    KT = HID // 128  # 16 contraction k-tiles for projections
    SC = 512         # s-chunk width
    NSC = S // SC    # 4
    TT = S // 128    # 16 t-tiles

    with TileContext(nc) as tc:
      for _rep in range(repeat):
       with ExitStack() as ctx:
        # ---------------- persistent tiles ----------------
        persist = ctx.enter_context(tc.tile_pool(name="persist", bufs=1))
        kT_sb = persist.tile([128, S], bf16)          # k' transposed (2 kv heads)
        # v tiles: cols 0:64 vA | 64 onesA | 65:129 vB | 129 onesB
        v_sb = persist.tile([128, TT, 130], bf16)
        rot_sb = persist.tile([128, 128], bf16)
        mask_sb = persist.tile([128, 256], f32r)      # bc outer-product rows
        cos_sb = persist.tile([128, S], f32)
        sin_sb = persist.tile([128, S], f32)
        wq_sb = persist.tile([128, KT, 512], bf16)
        wo_sb = persist.tile([128, 4, HID], bf16)

        nc.sync.dma_start(out=cos_sb, in_=cosT2[:, :])
        nc.sync.dma_start(out=sin_sb, in_=sinT2[:, :])
        nc.sync.dma_start(out=rot_sb, in_=rotm[:, :])
        nc.sync.dma_start(out=wq_sb, in_=wqT.rearrange("(t p) e -> p t e", p=128))
        nc.sync.dma_start(out=wo_sb, in_=woT.rearrange("(t p) h -> p t h", p=128))

        nc.vector.memset(v_sb, 1.0)  # ones col survives; data cols overwritten
        mask_f = persist.tile([128, 256], f32)
        nc.vector.memset(mask_f, 0.0)
        nc.vector.memset(mask_f[0:1, 0:64], 1.0)
        nc.vector.memset(mask_f[0:1, 192:256], 1.0)
        nc.vector.tensor_copy(mask_sb, mask_f)
        identb = persist.tile([64, 64], bf16)

        ropep = ctx.enter_context(tc.tile_pool(name="ropep", bufs=3))

        def rope_drain(ps, rot_ps, dst, sl, stage_eng):
            """ps/rot_ps: [128, SC] psum views. dst[:, sl] = ps*cos + (R@ps)*sin."""
            qsb = ropep.tile([128, SC], bf16, name="qsb", tag="qsb")
            stage_eng(qsb, ps)
            nc.tensor.matmul(rot_ps, rot_sb, qsb, start=True, stop=True)
            nc.vector.tensor_tensor(out=dst, in0=qsb, in1=cos_sb[:, sl], op=mult)
            shs = ropep.tile([128, SC], bf16, name="shs", tag="shs")
            nc.vector.tensor_tensor(out=shs, in0=rot_ps, in1=sin_sb[:, sl], op=mult)
            nc.vector.tensor_tensor(out=dst, in0=dst, in1=shs, op=add)

        # ---------------- prologue: K/V projections ----------------
        with ExitStack() as actx:
            pkv = actx.enter_context(tc.tile_pool(name="pkv", bufs=1))
            wk_sb = pkv.tile([128, KT, 128], bf16)
            wv_sb = pkv.tile([128, KT, 128], bf16)
            ident = pkv.tile([128, 128], f32)
            nc.sync.dma_start(out=wk_sb, in_=wkT.rearrange("(t p) e -> p t e", p=128))
            nc.sync.dma_start(out=wv_sb, in_=wvT.rearrange("(t p) e -> p t e", p=128))
            make_identity(nc, ident)
            nc.vector.tensor_copy(identb, ident[0:64, 0:64])

            hsp = actx.enter_context(tc.tile_pool(name="hsp", bufs=2))
            vstg = actx.enter_context(tc.tile_pool(name="vstg", bufs=2))
            pa = actx.enter_context(tc.tile_pool(name="pa", bufs=2, space="PSUM"))
            pb = actx.enter_context(tc.tile_pool(name="pb", bufs=2, space="PSUM"))

            for sc in range(NSC):
                sl = slice(sc * SC, (sc + 1) * SC)
                hs_sb = hsp.tile([128, KT, SC], bf16, name="hs_sb")
                nc.sync.dma_start(
                    out=hs_sb, in_=hsT[:, sl].rearrange("(t p) s -> p t s", p=128))
                kv = pa.tile([128, 2, SC], f32, name="kv", tag="pa")
                for ki in range(KT):
                    nc.tensor.matmul(kv[:, 0, :], wk_sb[:, ki, :], hs_sb[:, ki, :],
                                     start=ki == 0, stop=ki == KT - 1)
                for ki in range(KT):
                    nc.tensor.matmul(kv[:, 1, :], wv_sb[:, ki, :], hs_sb[:, ki, :],
                                     start=ki == 0, stop=ki == KT - 1)
                rb = pb.tile([128, 2, SC], f32, name="rb", tag="pb")
                rope_drain(kv[:, 0, :], rb[:, 0, :], kT_sb[:, sl], sl,
                           nc.scalar.copy)
                vt_sb = vstg.tile([128, SC], f32, name="vt_sb")
                nc.scalar.copy(vt_sb, kv[:, 1, :])
                for i in range(SC // 128):
                    tt = sc * (SC // 128) + i
                    tps = rb[:, 1, i * 128:(i + 1) * 128]
                    nc.tensor.transpose(tps, vt_sb[:, i * 128:(i + 1) * 128], ident)
                    nc.vector.tensor_copy(v_sb[:, tt, 0:64], tps[:, 0:64])
                    nc.vector.tensor_copy(v_sb[:, tt, 65:129], tps[:, 64:128])

        # ---------------- main pipeline over s-chunks ----------------
        hsp2 = ctx.enter_context(tc.tile_pool(name="hsp2", bufs=2))
        qstg = ctx.enter_context(tc.tile_pool(name="qstg", bufs=2))
        ptp = ctx.enter_context(tc.tile_pool(name="ptp", bufs=4))
        rrp = ctx.enter_context(tc.tile_pool(name="rrp", bufs=2))
        bcs = ctx.enter_context(tc.tile_pool(name="bcs", bufs=2))
        oTp = ctx.enter_context(tc.tile_pool(name="oTp", bufs=2))
        ogp = ctx.enter_context(tc.tile_pool(name="ogp", bufs=3))
        sp = ctx.enter_context(tc.tile_pool(name="sp", bufs=2, space="PSUM"))
        op = ctx.enter_context(tc.tile_pool(name="op", bufs=4, space="PSUM"))

        for sc in range(NSC):
            sl = slice(sc * SC, (sc + 1) * SC)
            hs_sb = hsp2.tile([128, KT, SC], bf16, name="hs2_sb")
            nc.sync.dma_start(
                out=hs_sb, in_=hsT[:, sl].rearrange("(t p) s -> p t s", p=128))
            qT_c = qstg.tile([128, 4, SC], bf16, name="qT_c")
            for e in range(4):
                qp = sp.tile([128, 2, SC], f32, name="qp", tag="sp")
                for ki in range(KT):
                    nc.tensor.matmul(qp[:, 0, :],
                                     wq_sb[:, ki, e * 128:(e + 1) * 128],
                                     hs_sb[:, ki, :],
                                     start=ki == 0, stop=ki == KT - 1)
                rope_drain(qp[:, 0, :], qp[:, 1, :], qT_c[:, e, :], sl,
                           nc.vector.tensor_copy)

            oT_c = oTp.tile([128, 4, SC], bf16, name="oT_c")
            for j in range(4):
                qA = qT_c[0:64, j, :]
                qB = qT_c[64:128, j, :]
                # A and B each [d(64) | rowsum] at psum partitions 0:65;
                # B's half is later shifted to partitions 64:128 via identb.
                oA = op.tile([128, SC], f32, name="oA", tag="op")
                oB = op.tile([128, SC], f32, name="oB", tag="op")

                def scores(tt):
                    ksl = slice(tt * 128, (tt + 1) * 128)
                    st_ = sp.tile([128, 2, SC], f32, name="st", tag="sp")
                    nc.tensor.matmul(st_[:, 0, :], kT_sb[0:64, ksl], qA,
                                     start=True, stop=True, tile_position=(0, 0))
                    nc.tensor.matmul(st_[:, 1, :], kT_sb[64:128, ksl], qB,
                                     start=True, stop=True, tile_position=(64, 0))
                    pt = ptp.tile([128, 2, SC], bf16, name="pt", tag="pt")
                    nc.scalar.activation(pt, st_, Exp, scale=0.125)
                    return pt

                def attnv(tt, pt):
                    st0 = tt == 0
                    sp0 = tt == TT - 1
                    nc.tensor.matmul(oA[0:65, :], v_sb[:, tt, 0:65], pt[:, 0, :],
                                     start=st0, stop=sp0)
                    nc.tensor.matmul(oB[0:65, :], v_sb[:, tt, 65:130],
                                     pt[:, 1, :], start=st0, stop=sp0)

                prev = scores(0)
                for tt in range(1, TT):
                    cur = scores(tt)
                    attnv(tt - 1, prev)
                    prev = cur
                attnv(TT - 1, prev)

                # normalize: rowsums (psum partition 64) -> partition 0 via
                # sbuf-dma, reciprocal, then base-0 outer-product broadcast
                rrst = rrp.tile([128, 2, SC], f32, name="rrst", tag="rrst")
                nc.vector.tensor_copy(rrst[64:65, 0, :], oA[64:65, :])
                nc.vector.tensor_copy(rrst[64:65, 1, :], oB[64:65, :])
                rr0 = rrp.tile([128, 2, SC], f32, name="rr0", tag="rr0")
                nc.sync.dma_start(out=rr0[0:1, :, :], in_=rrst[64:65, :, :])
                rr = rrp.tile([128, 2, SC], f32r, name="rr", tag="rr")
                with nc.allow_low_precision(reason="tf32 rowsum recip is plenty"):
                    nc.vector.reciprocal(rr[0:1, 0, :], rr0[0:1, 0, :])
                    nc.vector.reciprocal(rr[0:1, 1, :], rr0[0:1, 1, :])
                bc = op.tile([128, SC], f32, name="bc", tag="op")
                nc.tensor.matmul(bc, mask_sb[0:1, 0:128], rr[0:1, 0, :],
                                 start=True, stop=False)
                nc.tensor.matmul(bc, mask_sb[0:1, 128:256], rr[0:1, 1, :],
                                 start=False, stop=True)
                bc_sb = bcs.tile([128, SC], bf16, name="bc_sb")
                nc.vector.tensor_copy(bc_sb, bc)
                nc.vector.tensor_tensor(out=oT_c[0:64, j, :], in0=oA[0:64, :],
                                        in1=bc_sb[0:64, :], op=mult)
                # shift B's half to partitions 64:128 via identity matmul
                oBs = bcs.tile([64, SC], bf16, name="oBs", tag="oBs")
                nc.vector.tensor_copy(oBs, oB[0:64, :])
                oS = op.tile([128, SC], f32, name="oS", tag="op")
                nc.tensor.matmul(oS[64:128, :], identb, oBs, start=True, stop=True)
                nc.vector.tensor_tensor(out=oT_c[64:128, j, :], in0=oS[64:128, :],
                                        in1=bc_sb[64:128, :], op=mult)

            # o_proj for this chunk
            for st in range(SC // 128):
                ssl = slice(st * 128, (st + 1) * 128)
                gsl = slice(sc * SC + st * 128, sc * SC + (st + 1) * 128)
                for hc in range(HID // SC):
                    hsl = slice(hc * SC, (hc + 1) * SC)
                    ops = op.tile([128, SC], f32, name="ops", tag="op")
                    for et in range(4):
                        nc.tensor.matmul(ops, oT_c[:, et, ssl],
                                         wo_sb[:, et, hsl],
                                         start=et == 0, stop=et == 3)
                    og = ogp.tile([128, SC], bf16, name="og")
                    nc.vector.tensor_copy(og, ops)
                    nc.sync.dma_start(out=out[gsl, hsl], in_=og)

    nc.finalize()
    return nc


def _get_nc():
    if "nc" not in _NC_CACHE:
        _NC_CACHE["nc"] = _build_nc()
    return _NC_CACHE["nc"]


def _rot_matrix():
    # R @ q = rotate_half(q) per 64-block: R[i, i+32] = -1 (i%64<32),
    # R[i, i-32] = +1 (i%64>=32). Device needs lhsT = R.T.
    R = np.zeros((128, 128), dtype=np.float32)
    for blk in (0, 64):
        for i in range(32):
            R[blk + i, blk + i + 32] = -1.0
            R[blk + 32 + i, blk + i] = 1.0
    return np.ascontiguousarray(R.T)


def _marshal(inputs):
    import ml_dtypes

    bf16 = ml_dtypes.bfloat16
    hs = np.asarray(inputs["hidden_states"], dtype=np.float32)
    cos = np.asarray(inputs["cos"], dtype=np.float32)
    sin = np.asarray(inputs["sin"], dtype=np.float32)
    Wq = np.asarray(inputs["Wq"], dtype=np.float32)
    Wk = np.asarray(inputs["Wk"], dtype=np.float32)
    Wv = np.asarray(inputs["Wv"], dtype=np.float32)
    Wo = np.asarray(inputs["Wo"], dtype=np.float32)

    def c(x):
        return np.ascontiguousarray(x.astype(bf16))

    rotm = c(_rot_matrix())
    in_maps = []
    for core in range(NCORES):
        b, kg = divmod(core, 4)
        gheads = [kg * 8 + l for l in PERM_LOCAL]
        kvh = [2 * kg, 2 * kg + 1]
        wqT = c(Wq.reshape(H, D, HID)[gheads].reshape(512, HID).T)
        wkT = c(Wk.reshape(KVH, D, HID)[kvh].reshape(128, HID).T)
        wvT = c(Wv.reshape(KVH, D, HID)[kvh].reshape(128, HID).T)
        woT = c(Wo.T.reshape(H, D, HID)[gheads].reshape(512, HID))
        hsT = c(hs[b].T)
        cosT = cos[b].T  # [64, S]
        sinT = sin[b].T
        cosT2 = np.ascontiguousarray(np.concatenate([cosT, cosT], axis=0))
        sinT2 = np.ascontiguousarray(np.concatenate([sinT, sinT], axis=0))
        in_maps.append({
            "hsT": hsT, "cosT2": cosT2, "sinT2": sinT2, "rotm": rotm,
            "wqT": wqT, "wkT": wkT, "wvT": wvT, "woT": woT,
        })
    return in_maps


def run(inputs, trace=False, trace_cores=None):
    from concourse.bass_utils import run_bass_kernel_spmd

    nc = _get_nc()
    in_maps = _marshal(inputs)
    res = run_bass_kernel_spmd(
        nc, in_maps, core_ids=list(range(NCORES)), trace=trace,
        trace_cores=trace_cores)
    outs = [np.asarray(res.results[i]["out"]).astype(np.float32)
            for i in range(NCORES)]
    final = np.zeros((B, S, HID), dtype=np.float32)
    for b in range(B):
        final[b] = outs[4 * b] + outs[4 * b + 1] + outs[4 * b + 2] + outs[4 * b + 3]
    return final, res


def kernel(**inputs):
    out, _ = run(inputs, trace=False)
    return out


# revision 20
# speedup vs baseline: 1.0015x; 1.0015x over previous
"""Trainium2 Bass kernel for GQA multi-head attention (B=2,S=2048,HID=2048,H=32,KVH=8,D=64).

Sharding: 8 cores = 2 (batch) x 4 (kv-head groups). Each core handles one batch
element and 2 kv heads (= 8 q heads), computes its partial o_proj output
(contracting only its 512 attention features), host sums 4 partials per batch.

Pipeline layout (all GEMM operands bf16, f32 PSUM accumulate):
  prologue: K/V projections + K RoPE + V transpose, hs streamed once per chunk
  per 512-token chunk: Q proj + RoPE -> per j (4 q-tile pairs):
      scores (row-tiled 64+64 on the PE for both heads) -> exp (scalar engine,
      1024-wide, bf16 out) -> attnV (head A psum partitions 0:65 with rowsum
      at 64, head B partitions 63:128 with rowsum at 63 -- ones column FIRST
      in B's lhsT so no cross-partition shuffle is needed afterwards)
      -> reciprocal + PE outer-product broadcast -> normalized oT (bf16)
    then o_proj for the chunk + bf16 partial out DMA.

Softmax: scores are O(10) so exp without max-subtraction is safe; rowsums ride
as ones columns in the attn@V weights.
"""

import sys

if "/opt/trn_rl_repo" not in sys.path:
    sys.path.insert(0, "/opt/trn_rl_repo")

import numpy as np

B, S, HID = 2, 2048, 2048
H, KVH, D = 32, 8, 64
NCORES = 8

PERM_LOCAL = [0, 4, 1, 5, 2, 6, 3, 7]

_NC_CACHE = {}


def _build_nc(repeat=1):
    import concourse.bass as bass
    import concourse.mybir as mybir
    from concourse import bacc
    from concourse.tile import TileContext
    from concourse.masks import make_identity
    from contextlib import ExitStack

    f32 = mybir.dt.float32
    f32r = mybir.dt.float32r
    bf16 = mybir.dt.bfloat16
    Exp = mybir.ActivationFunctionType.Exp
    mult = mybir.AluOpType.mult
    add = mybir.AluOpType.add

    nc = bacc.Bacc(None, target_bir_lowering=False)

    hsT = nc.declare_dram_parameter("hsT", [HID, S], bf16, isOutput=False)
    cosT2 = nc.declare_dram_parameter("cosT2", [128, S], f32, isOutput=False)
    sinT2 = nc.declare_dram_parameter("sinT2", [128, S], f32, isOutput=False)
    rotm = nc.declare_dram_parameter("rotm", [128, 128], bf16, isOutput=False)
    wqT = nc.declare_dram_parameter("wqT", [HID, 512], bf16, isOutput=False)
    wkT = nc.declare_dram_parameter("wkT", [HID, 128], bf16, isOutput=False)
    wvT = nc.declare_dram_parameter("wvT", [HID, 128], bf16, isOutput=False)
    woT = nc.declare_dram_parameter("woT", [512, HID], bf16, isOutput=False)
    out = nc.declare_dram_parameter("out", [S, HID], bf16, isOutput=True)

    KT = HID // 128  # 16 contraction k-tiles for projections
    SC = 512         # s-chunk width
    NSC = S // SC    # 4
    TT = S // 128    # 16 t-tiles

    with TileContext(nc) as tc:
      for _rep in range(repeat):
       with ExitStack() as ctx:
        # ---------------- persistent tiles ----------------
        persist = ctx.enter_context(tc.tile_pool(name="persist", bufs=1))
        kT_sb = persist.tile([128, S], bf16)          # k' transposed (2 kv heads)
        # v tiles: cols 0:64 vA | 64 onesA | 65:129 vB | 129 onesB
        v_sb = persist.tile([128, TT, 130], bf16)
        rot_sb = persist.tile([128, 128], bf16)
        mask_sb = persist.tile([128, 256], f32r)      # bc outer-product rows
        cos_sb = persist.tile([128, S], f32)
        sin_sb = persist.tile([128, S], f32)
        wq_sb = persist.tile([128, KT, 512], bf16)
        wo_sb = persist.tile([128, 4, HID], bf16)

        nc.sync.dma_start(out=cos_sb, in_=cosT2[:, :])
        nc.sync.dma_start(out=sin_sb, in_=sinT2[:, :])
        nc.sync.dma_start(out=rot_sb, in_=rotm[:, :])
        nc.sync.dma_start(out=wq_sb, in_=wqT.rearrange("(t p) e -> p t e", p=128))
        nc.sync.dma_start(out=wo_sb, in_=woT.rearrange("(t p) h -> p t h", p=128))

        nc.vector.memset(v_sb, 1.0)  # ones col survives; data cols overwritten
        mask_f = persist.tile([128, 256], f32)
        nc.vector.memset(mask_f, 0.0)
        nc.vector.memset(mask_f[64:65, 0:64], 1.0)
        nc.vector.memset(mask_f[64:65, 192:256], 1.0)
        nc.vector.tensor_copy(mask_sb, mask_f)
        identb = persist.tile([64, 64], bf16)

        ropep = ctx.enter_context(tc.tile_pool(name="ropep", bufs=3))

        def rope_drain(ps, rot_ps, dst, sl, stage_eng):
            """ps/rot_ps: [128, SC] psum views. dst[:, sl] = ps*cos + (R@ps)*sin."""
            qsb = ropep.tile([128, SC], bf16, name="qsb", tag="qsb")
            stage_eng(qsb, ps)
            nc.tensor.matmul(rot_ps, rot_sb, qsb, start=True, stop=True)
            nc.vector.tensor_tensor(out=dst, in0=qsb, in1=cos_sb[:, sl], op=mult)
            shs = ropep.tile([128, SC], bf16, name="shs", tag="shs")
            nc.vector.tensor_tensor(out=shs, in0=rot_ps, in1=sin_sb[:, sl], op=mult)
            nc.vector.tensor_tensor(out=dst, in0=dst, in1=shs, op=add)

        # ---------------- prologue: K/V projections ----------------
        with ExitStack() as actx:
            pkv = actx.enter_context(tc.tile_pool(name="pkv", bufs=1))
            wk_sb = pkv.tile([128, KT, 128], bf16)
            wv_sb = pkv.tile([128, KT, 128], bf16)
            ident = pkv.tile([128, 128], f32)
            nc.sync.dma_start(out=wk_sb, in_=wkT.rearrange("(t p) e -> p t e", p=128))
            nc.sync.dma_start(out=wv_sb, in_=wvT.rearrange("(t p) e -> p t e", p=128))
            make_identity(nc, ident)
            nc.vector.tensor_copy(identb, ident[0:64, 0:64])

            hsp = actx.enter_context(tc.tile_pool(name="hsp", bufs=2))
            vstg = actx.enter_context(tc.tile_pool(name="vstg", bufs=2))
            pa = actx.enter_context(tc.tile_pool(name="pa", bufs=2, space="PSUM"))
            pb = actx.enter_context(tc.tile_pool(name="pb", bufs=2, space="PSUM"))

            for sc in range(NSC):
                sl = slice(sc * SC, (sc + 1) * SC)
                hs_sb = hsp.tile([128, KT, SC], bf16, name="hs_sb")
                nc.sync.dma_start(
                    out=hs_sb, in_=hsT[:, sl].rearrange("(t p) s -> p t s", p=128))
                kv = pa.tile([128, 2, SC], f32, name="kv", tag="pa")
                for ki in range(KT):
                    nc.tensor.matmul(kv[:, 0, :], wk_sb[:, ki, :], hs_sb[:, ki, :],
                                     start=ki == 0, stop=ki == KT - 1)
                for ki in range(KT):
                    nc.tensor.matmul(kv[:, 1, :], wv_sb[:, ki, :], hs_sb[:, ki, :],
                                     start=ki == 0, stop=ki == KT - 1)
                rb = pb.tile([128, 2, SC], f32, name="rb", tag="pb")
                rope_drain(kv[:, 0, :], rb[:, 0, :], kT_sb[:, sl], sl,
                           nc.scalar.copy)
                vt_sb = vstg.tile([128, SC], f32, name="vt_sb")
                nc.scalar.copy(vt_sb, kv[:, 1, :])
                for i in range(SC // 128):
                    tt = sc * (SC // 128) + i
                    tps = rb[:, 1, i * 128:(i + 1) * 128]
                    nc.tensor.transpose(tps, vt_sb[:, i * 128:(i + 1) * 128], ident)
                    nc.vector.tensor_copy(v_sb[:, tt, 0:64], tps[:, 0:64])
                    nc.vector.tensor_copy(v_sb[:, tt, 65:129], tps[:, 64:128])

        # ---------------- main pipeline over s-chunks ----------------
        hsp2 = ctx.enter_context(tc.tile_pool(name="hsp2", bufs=2))
        qstg = ctx.enter_context(tc.tile_pool(name="qstg", bufs=2))
        ptp = ctx.enter_context(tc.tile_pool(name="ptp", bufs=4))
        rrp = ctx.enter_context(tc.tile_pool(name="rrp", bufs=2))
        bcs = ctx.enter_context(tc.tile_pool(name="bcs", bufs=2))
        oTp = ctx.enter_context(tc.tile_pool(name="oTp", bufs=2))
        ogp = ctx.enter_context(tc.tile_pool(name="ogp", bufs=3))
        sp = ctx.enter_context(tc.tile_pool(name="sp", bufs=2, space="PSUM"))
        op = ctx.enter_context(tc.tile_pool(name="op", bufs=4, space="PSUM"))

        for sc in range(NSC):
            sl = slice(sc * SC, (sc + 1) * SC)
            hs_sb = hsp2.tile([128, KT, SC], bf16, name="hs2_sb")
            nc.sync.dma_start(
                out=hs_sb, in_=hsT[:, sl].rearrange("(t p) s -> p t s", p=128))
            qT_c = qstg.tile([128, 4, SC], bf16, name="qT_c")
            for e in range(4):
                qp = sp.tile([128, 2, SC], f32, name="qp", tag="sp")
                for ki in range(KT):
                    nc.tensor.matmul(qp[:, 0, :],
                                     wq_sb[:, ki, e * 128:(e + 1) * 128],
                                     hs_sb[:, ki, :],
                                     start=ki == 0, stop=ki == KT - 1)
                rope_drain(qp[:, 0, :], qp[:, 1, :], qT_c[:, e, :], sl,
                           nc.vector.tensor_copy)

            oT_c = oTp.tile([128, 4, SC], bf16, name="oT_c")
            pending = [None]

            def attention_j(j):
                qA = qT_c[0:64, j, :]
                qB = qT_c[64:128, j, :]
                # A and B each [d(64) | rowsum] at psum partitions 0:65;
                # B's half is later shifted to partitions 64:128 via identb.
                oA = op.tile([128, SC], f32, name="oA", tag="op")
                oB = op.tile([128, SC], f32, name="oB", tag="op")

                def scores(tt):
                    ksl = slice(tt * 128, (tt + 1) * 128)
                    st_ = sp.tile([128, 2, SC], f32, name="st", tag="sp")
                    nc.tensor.matmul(st_[:, 0, :], kT_sb[0:64, ksl], qA,
                                     start=True, stop=True, tile_position=(0, 0))
                    nc.tensor.matmul(st_[:, 1, :], kT_sb[64:128, ksl], qB,
                                     start=True, stop=True, tile_position=(64, 0))
                    pt = ptp.tile([128, 2, SC], bf16, name="pt", tag="pt")
                    nc.scalar.activation(pt, st_, Exp, scale=0.125)
                    return pt

                def attnv(tt, pt):
                    st0 = tt == 0
                    sp0 = tt == TT - 1
                    nc.tensor.matmul(oA[0:65, :], v_sb[:, tt, 0:65], pt[:, 0, :],
                                     start=st0, stop=sp0)
                    nc.tensor.matmul(oB[0:65, :], v_sb[:, tt, 65:130],
                                     pt[:, 1, :], start=st0, stop=sp0)

                prev = scores(0)
                cur = scores(1)
                # normalization tail of the previous j runs here, hidden
                # behind this j's first scores
                if pending[0] is not None:
                    pending[0]()
                    pending[0] = None
                attnv(0, prev)
                prev = cur
                for tt in range(2, TT):
                    cur = scores(tt)
                    attnv(tt - 1, prev)
                    prev = cur
                attnv(TT - 1, prev)

                def normalize():
                    # stage B's half early (only dep: oB accumulation done)
                    oBs = bcs.tile([64, SC], bf16, name="oBs", tag="oBs")
                    nc.vector.tensor_copy(oBs, oB[0:64, :])
                    rr = rrp.tile([128, 2, SC], f32r, name="rr", tag="rr")
                    with nc.allow_low_precision(reason="tf32 recip is plenty"):
                        nc.vector.reciprocal(rr[64:65, 0, :], oA[64:65, :])
                        nc.vector.reciprocal(rr[64:65, 1, :], oB[64:65, :])
                    bc = op.tile([128, SC], f32, name="bc", tag="op")
                    nc.tensor.matmul(bc, mask_sb[64:65, 0:128], rr[64:65, 0, :],
                                     start=True, stop=False)
                    nc.tensor.matmul(bc, mask_sb[64:65, 128:256], rr[64:65, 1, :],
                                     start=False, stop=True)
                    oS = op.tile([128, SC], f32, name="oS", tag="op")
                    nc.tensor.matmul(oS[64:128, :], identb, oBs,
                                     start=True, stop=True)
                    bc_sb = bcs.tile([128, SC], bf16, name="bc_sb")
                    nc.vector.tensor_copy(bc_sb, bc)
                    nc.vector.tensor_tensor(out=oT_c[0:64, j, :], in0=oA[0:64, :],
                                            in1=bc_sb[0:64, :], op=mult)
                    nc.vector.tensor_tensor(out=oT_c[64:128, j, :],
                                            in0=oS[64:128, :],
                                            in1=bc_sb[64:128, :], op=mult)

                pending[0] = normalize

            for j in range(4):
                attention_j(j)
            pending[0]()
            pending[0] = None

            # o_proj for this chunk
            for st in range(SC // 128):
                ssl = slice(st * 128, (st + 1) * 128)
                gsl = slice(sc * SC + st * 128, sc * SC + (st + 1) * 128)
                for hc in range(HID // SC):
                    hsl = slice(hc * SC, (hc + 1) * SC)
                    ops = op.tile([128, SC], f32, name="ops", tag="op")
                    for et in range(4):
                        nc.tensor.matmul(ops, oT_c[:, et, ssl],
                                         wo_sb[:, et, hsl],
                                         start=et == 0, stop=et == 3)
                    og = ogp.tile([128, SC], bf16, name="og")
                    nc.vector.tensor_copy(og, ops)
                    nc.sync.dma_start(out=out[gsl, hsl], in_=og)

    nc.finalize()
    return nc


def _get_nc():
    if "nc" not in _NC_CACHE:
        _NC_CACHE["nc"] = _build_nc()
    return _NC_CACHE["nc"]


def _rot_matrix():
    # R @ q = rotate_half(q) per 64-block: R[i, i+32] = -1 (i%64<32),
    # R[i, i-32] = +1 (i%64>=32). Device needs lhsT = R.T.
    R = np.zeros((128, 128), dtype=np.float32)
    for blk in (0, 64):
        for i in range(32):
            R[blk + i, blk + i + 32] = -1.0
            R[blk + 32 + i, blk + i] = 1.0
    return np.ascontiguousarray(R.T)


def _marshal(inputs):
    import ml_dtypes

    bf16 = ml_dtypes.bfloat16
    hs = np.asarray(inputs["hidden_states"], dtype=np.float32)
    cos = np.asarray(inputs["cos"], dtype=np.float32)
    sin = np.asarray(inputs["sin"], dtype=np.float32)
    Wq = np.asarray(inputs["Wq"], dtype=np.float32)
    Wk = np.asarray(inputs["Wk"], dtype=np.float32)
    Wv = np.asarray(inputs["Wv"], dtype=np.float32)
    Wo = np.asarray(inputs["Wo"], dtype=np.float32)

    def c(x):
        return np.ascontiguousarray(x.astype(bf16))

    rotm = c(_rot_matrix())
    in_maps = []
    for core in range(NCORES):
        b, kg = divmod(core, 4)
        gheads = [kg * 8 + l for l in PERM_LOCAL]
        kvh = [2 * kg, 2 * kg + 1]
        wqT = c(Wq.reshape(H, D, HID)[gheads].reshape(512, HID).T)
        wkT = c(Wk.reshape(KVH, D, HID)[kvh].reshape(128, HID).T)
        wvT = c(Wv.reshape(KVH, D, HID)[kvh].reshape(128, HID).T)
        woT = c(Wo.T.reshape(H, D, HID)[gheads].reshape(512, HID))
        hsT = c(hs[b].T)
        cosT = cos[b].T  # [64, S]
        sinT = sin[b].T
        cosT2 = np.ascontiguousarray(np.concatenate([cosT, cosT], axis=0))
        sinT2 = np.ascontiguousarray(np.concatenate([sinT, sinT], axis=0))
        in_maps.append({
            "hsT": hsT, "cosT2": cosT2, "sinT2": sinT2, "rotm": rotm,
            "wqT": wqT, "wkT": wkT, "wvT": wvT, "woT": woT,
        })
    return in_maps


def run(inputs, trace=False, trace_cores=None):
    from concourse.bass_utils import run_bass_kernel_spmd

    nc = _get_nc()
    in_maps = _marshal(inputs)
    res = run_bass_kernel_spmd(
        nc, in_maps, core_ids=list(range(NCORES)), trace=trace,
        trace_cores=trace_cores)
    outs = [np.asarray(res.results[i]["out"]).astype(np.float32)
            for i in range(NCORES)]
    final = np.zeros((B, S, HID), dtype=np.float32)
    for b in range(B):
        final[b] = outs[4 * b] + outs[4 * b + 1] + outs[4 * b + 2] + outs[4 * b + 3]
    return final, res


def kernel(**inputs):
    out, _ = run(inputs, trace=False)
    return out
